# revision 7
# baseline (speedup 1.0000x reference)
"""Distributed 2-hop GCN (scatter-mean propagation) for 8 Trainium2 NeuronCores.

Math: h0 = x @ W.T + b; two hops of h <- segment_mean(h[dst], src) over
edges with self loops (bias folded into h0 -- mean-propagation is linear).

Per-core scheme (targets sharded by node id; ~400k edges + 12.5k self
loops per core). Gathers/scatters use the stock Q7 ucode (dma_gather /
dma_scatter_add, int16 indices, 256B elements) -- one instruction per
~4096 rows instead of one 128-row indirect DMA per tile:

  * the hop table is fp32 [100352, 64] (256B rows, required stride) built
    from a bf16 AllGather via one contiguous cast-DMA (expand).
  * edges are packed per (source-window, target) into tiles of 128 edge
    slots x <=16 target segments; windows are 25088 table rows so gather
    indices fit int16. Chunks = 32 window-pure tiles; ONE dma_gather per
    chunk fetches 4096 rows into a [128, 32, 64] tile.
  * per tile one PE matmul msg^T @ sel -> PSUM bank [64, 512] (msg cast
    to bf16 on DVE); 4 PE transposes + ACT x (1/deg) per chunk fill a
    [128, 4, 64] fp32 slot block of a batch tile.
  * ONE dma_scatter_add per <=8-chunk batch accumulates slot rows into
    the target rows (fp32 CCE add; window partials sum in HBM; the dest
    is pre-zeroed, dummy slots carry scale 0 onto a padding row).
  * hop1 dest h1f (fp32) is cast to bf16, AllGathered, expanded to the
    hop2 table; hop2 scatter-adds fp32 into the (zero-donated) output.
"""

import numpy as np

N = 100000
NCORES = 8
SHARD = N // NCORES           # 12500
GROUPS = 98                   # 98 * 128 = 12544
SHARD_PAD = GROUPS * 128      # 12544
F = 64                        # output features
IN = 128                      # input features
P = 128                       # edge slots per tile
K = 16                        # target segments per tile
TB = 32                       # tiles per chunk (TB*K = 512 slots/chunk)
GQ = 8                        # tiles per dma_gather (1024 idxs: SWDGE ring cap)
SB = 2                        # max chunks per scatter batch (1024 idxs)
NWIN = 4
WINROWS = NCORES * SHARD_PAD // NWIN   # 25088
NUM_LAYERS = 2
NQ = 4                        # SWDGE queues
DUMMY = SHARD                 # dummy slots land on a padding row


# ----------------------------------------------------------------------------
# host-side preprocessing (indices only -- no tensor math)
# ----------------------------------------------------------------------------

def _pack_window(tgt_loc, src_loc):
    """Edges of one (core, window): local targets tgt_loc, window-local
    source rows src_loc. Greedy big/small pack into tiles of P edge slots /
    K target segments. Returns gloc [T,P] int16, seg [T,P] f32,
    slot_tgt [T*K] int64 (local target per slot, -1 dummy)."""
    order = np.argsort(tgt_loc, kind="stable")
    s = tgt_loc[order]
    d = src_loc[order]
    tgt_ids, seg_starts = np.unique(s, return_index=True)
    seg_ends = np.append(seg_starts[1:], len(s))
    degs = seg_ends - seg_starts
    assert degs.max() <= P, f"window segment {degs.max()} exceeds tile size"
    bydeg = np.argsort(degs, kind="stable")
    lo, hi = 0, len(bydeg) - 1
    tiles_g, tiles_s, slot_tgt = [], [], []
    cur_g = np.zeros(P, np.int16)
    cur_s = np.full(P, K, np.float32)
    fill = 0
    slots = 0

    def flush():
        nonlocal fill, slots
        slot_tgt.extend([-1] * (K - slots))
        tiles_g.append(cur_g.copy())
        tiles_s.append(cur_s.copy())
        cur_g[:] = 0
        cur_s[:] = K
        fill = 0
        slots = 0

    def put(ti):
        nonlocal fill, slots
        deg = int(degs[ti])
        a = int(seg_starts[ti])
        cur_g[fill:fill + deg] = d[a:a + deg]
        cur_s[fill:fill + deg] = slots
        slot_tgt.append(int(tgt_ids[ti]))
        fill += deg
        slots += 1

    while lo <= hi:
        if fill + int(degs[bydeg[hi]]) <= P and slots < K:
            put(int(bydeg[hi]))
            hi -= 1
        elif fill + int(degs[bydeg[lo]]) <= P and slots < K:
            put(int(bydeg[lo]))
            lo += 1
        else:
            flush()
    if slots or fill:
        flush()
    return (np.stack(tiles_g), np.stack(tiles_s),
            np.array(slot_tgt, np.int64))


def _node_to_row(v):
    """node id -> row in the (padded-shard) node table."""
    c = v // SHARD
    return c * SHARD_PAD + (v - c * SHARD)


def _wrap16(flat):
    """flat int idx list (len % 16 == 0) -> [128, len/16] int16."""
    w = np.asarray(flat, np.int16).reshape(-1, 16).T
    return np.tile(w, (8, 1))


def _prepare(edge_index):
    tgt = np.asarray(edge_index[0], np.int64)   # scatter targets
    src = np.asarray(edge_index[1], np.int64)   # gather sources
    loops = np.arange(N, dtype=np.int64)
    tgt_all = np.concatenate([tgt, loops])
    src_all = np.concatenate([src, loops])
    deg = np.bincount(tgt_all, minlength=N).astype(np.float64)  # >= 1
    grow = _node_to_row(src_all)
    win_all = grow // WINROWS
    loc_all = grow - win_all * WINROWS

    packed = [[None] * NWIN for _ in range(NCORES)]
    for c in range(NCORES):
        base = c * SHARD
        mc = (tgt_all >= base) & (tgt_all < base + SHARD)
        tc, wc, lc = tgt_all[mc] - base, win_all[mc], loc_all[mc]
        for w in range(NWIN):
            m = wc == w
            packed[c][w] = _pack_window(tc[m], lc[m].astype(np.int16))

    # common per-window chunk counts across cores (the chunk->window map is
    # baked into the SPMD program)
    nchunk_w = [
        -(-max(packed[c][w][0].shape[0] for c in range(NCORES)) // TB)
        for w in range(NWIN)
    ]
    nchunk = sum(nchunk_w)

    # program plan: per chunk its window; scatter batches as (c0, c1) runs
    chunk_win = []
    for w in range(NWIN):
        chunk_win.extend([w] * nchunk_w[w])
    batches = []
    base = 0
    for w in range(NWIN):
        c0 = base
        while c0 < base + nchunk_w[w]:
            c1 = min(c0 + SB, base + nchunk_w[w])
            batches.append((c0, c1))
            c0 = c1
        base += nchunk_w[w]
    plan = dict(nchunk=nchunk, chunk_win=chunk_win, batches=batches)

    cores = []
    for c in range(NCORES):
        degc = deg[c * SHARD:(c + 1) * SHARD]
        g_all = np.zeros((nchunk * TB, P), np.int16)
        s_all = np.full((nchunk * TB, P), K, np.float32)
        st_all = np.full((nchunk * TB, K), -1, np.int64)
        base_t = 0
        for w in range(NWIN):
            g, sgm, st = packed[c][w]
            t = g.shape[0]
            g_all[base_t:base_t + t] = g
            s_all[base_t:base_t + t] = sgm
            st_all[base_t:base_t + t] = st.reshape(t, K)
            base_t += nchunk_w[w] * TB

        gidx16 = np.concatenate(
            [_wrap16(g_all[k * TB:(k + 1) * TB].ravel())
             for k in range(nchunk)], axis=1)                  # [128, nchunk*256]
        seg = np.ascontiguousarray(s_all.T)                    # [128, nchunk*TB]
        st_c = st_all.reshape(nchunk, TB * K)                  # slot s = j*K+o
        sidx16 = np.concatenate(
            [_wrap16(np.where(st_c[k] >= 0, st_c[k], DUMMY))
             for k in range(nchunk)], axis=1)                  # [128, nchunk*32]
        scale_s = np.where(
            st_c >= 0, 1.0 / np.maximum(degc[np.clip(st_c, 0, SHARD - 1)], 1.0),
            0.0)                                               # [nchunk, 512]
        # quarter q, partition p -> slot q*128+p
        cscale = (scale_s.reshape(nchunk, 4, 128).transpose(2, 0, 1)
                  .reshape(128, nchunk * 4).astype(np.float32).copy())
        cores.append(dict(gidx16=np.ascontiguousarray(gidx16),
                          seg=seg,
                          sidx16=np.ascontiguousarray(sidx16),
                          cscale=cscale))
    return cores, plan


# ----------------------------------------------------------------------------
# bass program
# ----------------------------------------------------------------------------

def _build(plan, repeat=1):
    import concourse.bacc as bacc
    import concourse.bass as bass
    import concourse.mybir as mybir
    from concourse.tile import TileContext
    from concourse.library_config import mlp

    nchunk = plan["nchunk"]
    chunk_win = plan["chunk_win"]
    batches = plan["batches"]

    dt = mybir.dt
    nc = bacc.Bacc("TRN2", num_swdge_queues=NQ)

    xT = nc.dram_tensor("xT", [IN, SHARD_PAD], dt.bfloat16, kind="ExternalInput")
    wt = nc.dram_tensor("wt", [IN, F], dt.bfloat16, kind="ExternalInput")
    brow = nc.dram_tensor("brow", [1, F], dt.bfloat16, kind="ExternalInput")
    id64 = nc.dram_tensor("id64", [64, 64], dt.float32, kind="ExternalInput")
    iota = nc.dram_tensor("iota", [128, TB * K], dt.float32, kind="ExternalInput")
    gidx16 = nc.dram_tensor("gidx16", [128, nchunk * TB * P // 16], dt.int16,
                            kind="ExternalInput")
    seg = nc.dram_tensor("seg", [128, nchunk * TB], dt.float32,
                         kind="ExternalInput")
    sidx16 = nc.dram_tensor("sidx16", [128, nchunk * TB * K // 16], dt.int16,
                            kind="ExternalInput")
    cscale = nc.dram_tensor("cscale", [128, nchunk * 4], dt.float32,
                            kind="ExternalInput")
    zf32 = nc.dram_tensor("zf32", [SHARD_PAD, F], dt.float32,
                          kind="ExternalInput")
    out = nc.dram_tensor("out", [SHARD_PAD, F], dt.float32, kind="ExternalOutput")

    h_cmp = [nc.dram_tensor(f"h{i}_cmp", [SHARD_PAD, F], dt.bfloat16)
             for i in range(NUM_LAYERS)]
    h_ag = [nc.dram_tensor(f"h{i}_ag", [NCORES * SHARD_PAD, F], dt.bfloat16,
                           addr_space="Shared")
            for i in range(NUM_LAYERS)]
    h_tab = [nc.dram_tensor(f"h{i}_tab", [NCORES * SHARD_PAD, F], dt.float32)
             for i in range(NUM_LAYERS)]
    h1f = nc.dram_tensor("h1f", [SHARD_PAD, F], dt.float32)

    with TileContext(nc) as tc:
        import contextlib
        with contextlib.ExitStack() as ctx:
            cpool = ctx.enter_context(tc.tile_pool(name="consts", bufs=1))
            gpool = ctx.enter_context(tc.tile_pool(name="gidx", bufs=4))
            mpool = ctx.enter_context(tc.tile_pool(name="msgs", bufs=4))
            bpool = ctx.enter_context(tc.tile_pool(name="msgb", bufs=3))
            spool = ctx.enter_context(tc.tile_pool(name="sel", bufs=3))
            opool = ctx.enter_context(tc.tile_pool(name="outs", bufs=3))
            tpool = ctx.enter_context(tc.tile_pool(name="obt", bufs=2))
            ppool = ctx.enter_context(tc.tile_pool(name="psum", bufs=2, space="PSUM"))
            qpool = ctx.enter_context(tc.tile_pool(name="psum_t", bufs=4, space="PSUM"))
            hpool = ctx.enter_context(tc.tile_pool(name="psum_h", bufs=2, space="PSUM"))

            nc.gpsimd.load_library(mlp)

            _qn = [0]

            def _gq():
                q = _qn[0] % 3
                _qn[0] += 1
                return q

            # ---- persistent SBUF state ----
            xT_sb = cpool.tile([IN, SHARD_PAD], dt.bfloat16)
            nc.sync.dma_start(out=xT_sb[:], in_=xT[:])
            wt_sb = cpool.tile([IN, F], dt.bfloat16)
            nc.sync.dma_start(out=wt_sb[:], in_=wt[:])
            brow_sb = cpool.tile([1, F], dt.bfloat16)
            nc.sync.dma_start(out=brow_sb[:], in_=brow[:])
            id64_sb = cpool.tile([64, 64], dt.float32)
            nc.sync.dma_start(out=id64_sb[:], in_=id64[:])
            one_sb = cpool.tile([1, 128], dt.bfloat16)
            nc.vector.memset(one_sb[:], 1.0)
            iota_sb = cpool.tile([128, TB * K], dt.float32)
            nc.sync.dma_start(out=iota_sb[:], in_=iota[:])
            seg_sb = cpool.tile([128, nchunk * TB], dt.float32)
            nc.sync.dma_start(out=seg_sb[:], in_=seg[:])
            sidx_sb = cpool.tile([128, nchunk * TB * K // 16], dt.int16)
            nc.sync.dma_start(out=sidx_sb[:], in_=sidx16[:])
            cscale_sb = cpool.tile([128, nchunk * 4], dt.float32)
            nc.sync.dma_start(out=cscale_sb[:], in_=cscale[:])
            h0_all = cpool.tile([128, GROUPS * F], dt.bfloat16)

            def h0_stage(_r):
                """h0 = x @ W.T + b  -> h0_all (bf16) -> h_cmp[0]"""
                for g in range(GROUPS):
                    hp = hpool.tile([128, F], dt.float32, name=f"hp{_r}_{g}",
                                    tag="psum", space="PSUM")
                    nc.tensor.matmul(out=hp[:], lhsT=xT_sb[:, g * 128:(g + 1) * 128],
                                     rhs=wt_sb[:], start=True, stop=False)
                    nc.tensor.matmul(out=hp[:], lhsT=one_sb[:], rhs=brow_sb[:],
                                     start=False, stop=True)
                    nc.scalar.activation(
                        out=h0_all[:, g * F:(g + 1) * F], in_=hp[:],
                        func=mybir.ActivationFunctionType.Copy)
                nc.sync.dma_start(
                    out=h_cmp[0][:].rearrange("(g p) f -> p g f", p=128),
                    in_=h0_all[:].rearrange("p (g f) -> p g f", f=F))

            def allgather(i):
                nc.gpsimd.collective_compute(
                    "AllGather",
                    mybir.AluOpType.bypass,
                    ins=[h_cmp[i].ap()],
                    outs=[h_ag[i].ap()],
                    replica_groups=[list(range(NCORES))],
                )

            def expand(i):
                # bf16 -> fp32 contiguous cast-DMA builds the gather table
                nc.gpsimd.dma_start(out=h_tab[i][:], in_=h_ag[i][:])

            def hop(i, _r):
                """h_tab[i] -> h1f (fp32 add) or out (fp32 add, last)"""
                last = i == NUM_LAYERS - 1
                dest = out if last else h1f
                for (c0, c1) in batches:
                    bs = c1 - c0
                    obt = tpool.tile([128, 4 * SB, F], dt.float32,
                                     name=f"obt{i}r{_r}_{c0}", tag="obt")
                    for k in range(c0, c1):
                        kk = k - c0
                        w = chunk_win[k]
                        gt = gpool.tile([128, TB * P // 16], dt.int16,
                                        name=f"gt{i}r{_r}_{k}", tag="gt")
                        nc.sync.dma_start(
                            out=gt[:],
                            in_=gidx16[:, k * (TB * P // 16):(k + 1) * (TB * P // 16)])
                        msg = mpool.tile([128, TB, F], dt.float32,
                                         name=f"msg{i}r{_r}_{k}", tag="msg")
                        for t0 in range(0, TB, GQ):
                            nc.gpsimd.dma_gather(
                                msg[:, t0:t0 + GQ, :],
                                h_tab[i][w * WINROWS:(w + 1) * WINROWS, :],
                                gt[:, t0 * P // 16:(t0 + GQ) * P // 16],
                                GQ * P, GQ * P, F, queue_num=_gq())
                        msgb = bpool.tile([128, TB, F], dt.bfloat16,
                                          name=f"msgb{i}r{_r}_{k}", tag="msgb")
                        nc.vector.tensor_copy(out=msgb[:], in_=msg[:])
                        sel = spool.tile([128, TB, K], dt.bfloat16,
                                         name=f"sel{i}r{_r}_{k}", tag="sel")
                        nc.vector.tensor_tensor(
                            out=sel[:],
                            in0=seg_sb[:, k * TB:(k + 1) * TB]
                                .rearrange("p (t o) -> p t o", o=1)
                                .to_broadcast([128, TB, K]),
                            in1=iota_sb[:].rearrange("p (t o) -> p t o", o=K),
                            op=mybir.AluOpType.is_equal,
                        )
                        bank = ppool.tile([64, TB * K], dt.float32,
                                          name=f"bk{i}r{_r}_{k}", tag="bank",
                                          space="PSUM")
                        for j in range(TB):
                            nc.tensor.matmul(
                                out=bank[:, j * K:(j + 1) * K],
                                lhsT=msgb[:, j, :],
                                rhs=sel[:, j, :],
                                start=True, stop=True,
                            )
                        bsb = opool.tile([64, TB * K], dt.float32,
                                         name=f"bs{i}r{_r}_{k}", tag="bsb")
                        nc.vector.tensor_copy(out=bsb[:], in_=bank[:])
                        for q in range(4):
                            tps = qpool.tile([128, 64], dt.float32,
                                             name=f"tp{i}r{_r}_{k}_{q}", tag="tps",
                                             space="PSUM")
                            nc.tensor.transpose(out=tps[:],
                                                in_=bsb[:, q * 128:(q + 1) * 128],
                                                identity=id64_sb[:])
                            nc.scalar.activation(
                                out=obt[:, kk * 4 + q, :], in_=tps[:],
                                func=mybir.ActivationFunctionType.Copy,
                                scale=cscale_sb[:, k * 4 + q:k * 4 + q + 1])
                    nidx = bs * TB * K
                    nc.gpsimd.dma_scatter_add(
                        dest[:], obt[:, :4 * bs, :],
                        sidx_sb[:, c0 * TB * K // 16:c1 * TB * K // 16],
                        nidx, nidx, F, queue_num=3)

            for _r in range(repeat):
                h0_stage(_r)
                allgather(0)
                expand(0)
                # pre-zero the hop-1 accumulation dest (overlaps hop 1)
                nc.sync.dma_start(out=h1f[:], in_=zf32[:])
                hop(0, _r)
                # fp32 -> bf16 contiguous cast for the second AllGather
                nc.gpsimd.dma_start(out=h_cmp[1][:], in_=h1f[:])
                allgather(1)
                expand(1)
                hop(1, _r)

    nc.compile()
    return nc


# ----------------------------------------------------------------------------
# entry point
# ----------------------------------------------------------------------------

def _make_in_maps(x, W, b, cores):
    from ml_dtypes import bfloat16

    x = np.asarray(x, np.float32)
    W = np.asarray(W, np.float32)
    b = np.asarray(b, np.float32)
    iota = np.tile(np.arange(K, dtype=np.float32), (128, TB))
    zf32 = np.zeros((SHARD_PAD, F), np.float32)
    in_maps = []
    for c in range(NCORES):
        xs = np.zeros((SHARD_PAD, IN), np.float32)
        xs[:SHARD] = x[c * SHARD:(c + 1) * SHARD]
        in_maps.append({
            "xT": np.ascontiguousarray(xs.T).astype(bfloat16),
            "wt": np.ascontiguousarray(W.T).astype(bfloat16),
            "brow": b[None, :].astype(bfloat16),
            "id64": np.eye(64, dtype=np.float32),
            "iota": iota,
            "gidx16": cores[c]["gidx16"],
            "seg": cores[c]["seg"],
            "sidx16": cores[c]["sidx16"],
            "cscale": cores[c]["cscale"],
            "zf32": zf32,
        })
    return in_maps


def kernel(x, W, b, edge_index):
    from concourse import bass_utils

    x = np.asarray(x, np.float32)
    W = np.asarray(W, np.float32)
    b = np.asarray(b, np.float32)
    edge_index = np.asarray(edge_index)

    cores, plan = _prepare(edge_index)
    nc = _build(plan)
    in_maps = _make_in_maps(x, W, b, cores)

    res = bass_utils.run_bass_kernel_spmd(nc, in_maps, core_ids=list(range(NCORES)))
    outp = np.concatenate([res.results[c]["out"][:SHARD] for c in range(NCORES)],
                          axis=0)
    return outp.astype(np.float32)


if __name__ == "__main__":
    import importlib.util
    spec = importlib.util.spec_from_file_location("refmod", "/root/problem/reference.py")
    ref = importlib.util.module_from_spec(spec)
    spec.loader.exec_module(ref)
    inputs = {k: np.asarray(v) for k, v in ref.setup_inputs().items()}
    got = kernel(**inputs)
    print("kernel output", got.shape, got.dtype)
